# revision 12
# baseline (speedup 1.0000x reference)
"""DendriticMLP Trainium2 kernel (8 cores, hybrid sharding).

Strategy:
- W-path is data-parallel: batch 4096 -> 8 x 512, weights replicated,
  feature-major on-chip layout (h[unit(partition), batch(free)]), BatchNorm
  via a 16 KB AllReduce of per-unit (sum, sum_sq).
- Dendrite path is PAIR-SHARDED to halve its HBM stream (the binding limit):
  core c computes gates for unit-half j=c&1 (8 of 16 unit-tiles) over batch
  quarter i=c>>1 (1024 rows = the W-shards of cores 2i, 2i+1). Segment
  weights per core drop 402 MB -> 201 MB; each 512 KB chunk feeds 16 matmuls
  instead of 8, cutting dendrite DMA demand to ~150 GB/s, below delivered
  HBM bandwidth (~225-240 GB/s).
- Gates return to their batch owners via per-pair AllToAll (2 MB fp16,
  groups [[0,1],[2,3],[4,5],[6,7]]), hidden under the next compute phase.
  Issue order: W0, dend0, dend1, h0, W1, dend2(7 uts), h1, W2, dend2(last
  ut), h2, classifier -- every AllToAll/AllReduce is covered except the last
  AllToAll (~20 us).
- Dendrite matmuls run fp32r (HW truncates operands to 11 mantissa bits):
  the argmax over |activation| flips on near-tied segments under operand
  noise, rel-err ~ sqrt(noise): fp32r 1.6e-2 PASS, fp16 2.3e-2 FAIL,
  bf16 6.4e-2, fp8 0.25. W-path (selection-free) runs fp16: adds only
  ~1e-4 rel-err and halves that stream.
- sel = where(max_s d >= -min_s d, max, min) replaces argmax|.|-gather.
"""
import os
import sys
import types

sys.path.insert(0, "/opt/trn_rl_repo")

import numpy as np

import concourse.bass as bass
import concourse.mybir as mybir
import concourse.tile as tile
from concourse import bacc
from concourse.bass_utils import run_bass_kernel_spmd

B, D, H, S, OUT = 4096, 1024, 2048, 16, 1000
CORES = 8
BS = B // CORES            # 512 rows per core (W-path shard)
BQ = 2 * BS                # 1024 rows per dendrite quarter (pair of W-shards)
OUTP = 1024                # classifier outputs padded to 8*128
KT_D = D // 128            # 8 k-tiles for 1024-dim contractions
KT_H = H // 128            # 16 k-tiles for 2048-dim contractions
UT_H = H // 128            # 16 unit tiles per hidden layer
UT_HALF = UT_H // 2        # 8 unit tiles per dendrite half
UT_O = OUTP // 128         # 8 unit tiles for classifier
BN_EPS = 1e-5

F32 = mybir.dt.float32
F32R = mybir.dt.float32r
F16 = mybir.dt.float16
AX = mybir.AxisListType
ALU = mybir.AluOpType
ACTF = mybir.ActivationFunctionType

LAST_EXEC_NS = None
_CACHE = {}


def _install_ntff_shim():
    """Register antenv.axon_hooks so bass_utils can NTFF-profile under axon."""
    if "antenv.axon_hooks" in sys.modules:
        return
    try:
        from trn_agent_boot.trn_boot import _ntff_profile_via_ctypes

        hook = _ntff_profile_via_ctypes("/opt/axon/libaxon_pjrt.so")
        mod = types.ModuleType("antenv.axon_hooks")
        state = {"hook": hook}
        mod.set_axon_ntff_profile_hook = lambda h: state.__setitem__("hook", h)
        mod.get_axon_ntff_profile_hook = lambda: state["hook"]
        sys.modules["antenv.axon_hooks"] = mod
    except Exception:
        pass


def _build_nc():
    nc = bacc.Bacc("TRN2", target_bir_lowering=False, debug=False,
                   num_devices=CORES)

    xinT = nc.dram_tensor("xinT", [KT_D, 128, BS], F16, kind="ExternalInput").ap()
    ctxQ = nc.dram_tensor("ctxQ", [KT_D, 128, BQ], F32R, kind="ExternalInput").ap()
    wr0 = nc.dram_tensor("wr0", [UT_H, 128, KT_D, 128], F16, kind="ExternalInput").ap()
    wr1 = nc.dram_tensor("wr1", [UT_H, 128, KT_H, 128], F16, kind="ExternalInput").ap()
    wr2 = nc.dram_tensor("wr2", [UT_H, 128, KT_H, 128], F16, kind="ExternalInput").ap()
    wcr = nc.dram_tensor("wcr", [UT_O, 128, KT_H, 128], F16, kind="ExternalInput").ap()
    swr = [
        nc.dram_tensor(f"swr{i}", [UT_HALF, S, 128, KT_D, 128], F32R,
                       kind="ExternalInput").ap()
        for i in range(3)
    ]
    br = nc.dram_tensor("br", [128, 3 * UT_H], F32, kind="ExternalInput").ap()
    dmask = nc.dram_tensor("dmask", [128, 2], F32, kind="ExternalInput").ap()
    bcr = nc.dram_tensor("bcr", [128, UT_O], F32, kind="ExternalInput").ap()
    outT = nc.dram_tensor("outT", [UT_O, 128, BS], F32, kind="ExternalOutput").ap()

    wr = [wr0, wr1, wr2]
    PAIRS = [[2 * i, 2 * i + 1] for i in range(4)]

    with tile.TileContext(nc) as tc:
        with (
            tc.tile_pool(name="pers", bufs=1) as pers,
            tc.tile_pool(name="wblk", bufs=2) as wpool,
            tc.tile_pool(name="swp", bufs=3) as swpool,
            tc.tile_pool(name="work", bufs=2) as work,
            tc.tile_pool(name="works", bufs=1) as works,
            tc.tile_pool(name="ob", bufs=1) as opool,
            tc.tile_pool(name="pb", bufs=2, space="PSUM") as pb,
            tc.tile_pool(name="pd", bufs=6, space="PSUM") as pd,
            tc.tile_pool(name="dram", bufs=1, space="DRAM") as dram,
        ):
            xin_tiles = [pers.tile([128, BS], F16, tag=f"xin{k}", name=f"xin{k}")
                         for k in range(KT_D)]
            ctx_tiles = [pers.tile([128, BQ], F32R, tag=f"ctx{k}", name=f"ctx{k}")
                         for k in range(KT_D)]
            h_tiles = [pers.tile([128, BS], F16, tag=f"h{k}", name=f"h{k}")
                       for k in range(UT_H)]
            y_tiles = [pers.tile([128, BS], F16, tag=f"y{k}", name=f"y{k}")
                       for k in range(UT_H)]
            # 2 alternating slots: gate(L) is consumed before gate(L+2) exists
            gateT = [[pers.tile([128, BS], F16, tag=f"gt{L}_{k}", name=f"gt{L}_{k}")
                      for k in range(UT_H)] for L in range(2)]
            bias_sb = pers.tile([128, 3 * UT_H], F32, tag="bias_sb", name="bias_sb")
            bc_sb = pers.tile([128, UT_O], F32, tag="bc_sb", name="bc_sb")
            mask_sb = pers.tile([128, 2], F32, tag="mask_sb", name="mask_sb")
            stats_loc = [pers.tile([128, 2 * UT_H], F32, tag=f"stl{L}",
                                   name=f"stl{L}") for L in range(3)]
            stats_glob = [pers.tile([128, 2 * UT_H], F32, tag=f"stg{L}",
                                    name=f"stg{L}") for L in range(3)]
            scale_t = [pers.tile([128, UT_H], F32, tag=f"scale{L}",
                                 name=f"scale{L}") for L in range(3)]
            nbias_t = [pers.tile([128, UT_H], F32, tag=f"nbias{L}",
                                 name=f"nbias{L}") for L in range(3)]

            nc.sync.dma_start(bias_sb[:], br)
            nc.sync.dma_start(mask_sb[:], dmask)
            for k in range(KT_D):
                nc.sync.dma_start(xin_tiles[k][:], xinT[k])
            nc.sync.dma_start(bc_sb[:], bcr)

            def w_phase(layer):
                kt_in = KT_D if layer == 0 else KT_H
                in_tiles = xin_tiles if layer == 0 else h_tiles
                sl = stats_loc[layer]
                for ut in range(UT_H):
                    wchunk = wpool.tile([128, kt_in * 128], F16, tag="wblk",
                                        name=f"w{layer}_{ut}")
                    nc.sync.dma_start(
                        wchunk[:],
                        wr[layer][ut].rearrange("p a b -> p (a b)"),
                    )
                    ps = pb.tile([128, BS], F32, tag="yblk", name=f"yp{layer}_{ut}")
                    wv = wchunk[:]
                    for kt in range(kt_in):
                        nc.tensor.matmul(
                            ps[:],
                            wv[:, kt * 128:(kt + 1) * 128],
                            in_tiles[kt][:],
                            start=(kt == 0),
                            stop=(kt == kt_in - 1),
                        )
                    y = y_tiles[ut]
                    nc.scalar.activation(
                        y[:], ps[:], ACTF.Identity,
                        bias=bias_sb[:, layer * UT_H + ut:layer * UT_H + ut + 1],
                    )
                    nc.vector.tensor_reduce(
                        sl[:, ut:ut + 1], y[:], axis=AX.X, op=ALU.add)
                    # dummy out reuses the "ga" buffer; only the accum matters
                    sq = works.tile([128, BQ], F16, tag="ga", name=f"sq{layer}_{ut}")
                    nc.scalar.activation(
                        sq[:, :BS], y[:], ACTF.Square,
                        accum_out=sl[:, UT_H + ut:UT_H + ut + 1],
                    )
                # fire BN stats AllReduce; coefficients are computed later
                # (just before h_apply) so DVE isn't blocked on the collective
                bnc_in = dram.tile([128, 2 * UT_H], F32, tag=f"bin{layer}",
                                   name=f"bin{layer}")
                bnc_out = dram.tile([128, 2 * UT_H], F32, addr_space="Shared",
                                    tag=f"bout{layer}", name=f"bout{layer}")
                nc.sync.dma_start(bnc_in[:], sl[:])
                nc.gpsimd.collective_compute(
                    "AllReduce", ALU.add,
                    ins=[bnc_in.opt()],
                    outs=[bnc_out.opt()],
                    replica_groups=[list(range(CORES))],
                )
                nc.sync.dma_start(stats_glob[layer][:], bnc_out[:])

            def bn_coeffs(layer):
                sg = stats_glob[layer]
                scale = scale_t[layer]
                nbias = nbias_t[layer]
                mean = pers.tile([128, UT_H], F32, tag=f"mean{layer}",
                                 name=f"mean{layer}")
                var = pers.tile([128, UT_H], F32, tag=f"var{layer}",
                                name=f"var{layer}")
                msq = pers.tile([128, UT_H], F32, tag=f"msq{layer}",
                                name=f"msq{layer}")
                nc.vector.tensor_scalar_mul(mean[:], sg[:, 0:UT_H], 1.0 / B)
                nc.vector.tensor_scalar_mul(var[:], sg[:, UT_H:2 * UT_H], 1.0 / B)
                nc.vector.scalar_tensor_tensor(
                    out=msq[:], in0=mean[:], scalar=-1.0, in1=mean[:],
                    op0=ALU.mult, op1=ALU.mult,
                )
                nc.vector.tensor_tensor(var[:], var[:], msq[:], op=ALU.add)
                nc.vector.tensor_scalar_add(var[:], var[:], BN_EPS)
                nc.scalar.sqrt(scale[:], var[:])
                nc.vector.reciprocal(scale[:], scale[:])
                nc.vector.scalar_tensor_tensor(
                    out=nbias[:], in0=mean[:], scalar=-1.0, in1=scale[:],
                    op0=ALU.mult, op1=ALU.mult,
                )

            def dend_units(layer, ut_range, at_in):
                # gates for unit-half tiles ut_range over this core's batch
                # quarter; each sw chunk feeds 2 batch-tile psum chains.
                for utl in ut_range:
                    mx = work.tile([128, BQ], F32, tag="mx",
                                   name=f"mx{layer}_{utl}")
                    mn = work.tile([128, BQ], F32, tag="mn",
                                   name=f"mn{layer}_{utl}")
                    for s in range(S):
                        swc = swpool.tile([128, KT_D * 128], F32R, tag="sw",
                                          name=f"sw{layer}_{utl}_{s}")
                        nc.sync.dma_start(
                            swc[:],
                            swr[layer][utl, s].rearrange("p a b -> p (a b)"),
                        )
                        swv = swc[:]
                        for bt in range(2):
                            psd = pd.tile([128, BS], F32, tag="pd",
                                          name=f"pd{layer}_{utl}_{s}_{bt}")
                            for kt in range(KT_D):
                                nc.tensor.matmul(
                                    psd[:],
                                    swv[:, kt * 128:(kt + 1) * 128],
                                    ctx_tiles[kt][:, bt * BS:(bt + 1) * BS],
                                    start=(kt == 0),
                                    stop=(kt == KT_D - 1),
                                )
                            mxs = mx[:, bt * BS:(bt + 1) * BS]
                            mns = mn[:, bt * BS:(bt + 1) * BS]
                            if s == 0:
                                nc.scalar.copy(mxs, psd[:])
                                nc.vector.tensor_copy(mns, psd[:])
                            else:
                                nc.vector.tensor_tensor(mxs, mxs, psd[:],
                                                        op=ALU.max)
                                nc.vector.tensor_tensor(mns, mns, psd[:],
                                                        op=ALU.min)
                    # sel = where(mx >= -mn, mx, mn) -> gate = sigmoid(sel)
                    mask = works.tile([128, BQ], mybir.dt.uint8, tag="mask",
                                      name=f"mk{layer}_{utl}")
                    nc.vector.scalar_tensor_tensor(
                        out=mask[:], in0=mn[:], scalar=-1.0, in1=mx[:],
                        op0=ALU.mult, op1=ALU.is_le,
                    )
                    nc.vector.copy_predicated(mn[:], mask[:], mx[:])
                    g = works.tile([128, BQ], F16, tag="g", name=f"g{layer}_{utl}")
                    nc.scalar.activation(g[:], mn[:], ACTF.Sigmoid)
                    # mask-route: slots [0:8] carry half-A cores' gates, slots
                    # [8:16] half-B; the pair ReduceScatter-add concatenates.
                    ga = works.tile([128, BQ], F16, tag="ga", name=f"ga{layer}_{utl}")
                    gb = works.tile([128, BQ], F16, tag="gb", name=f"gb{layer}_{utl}")
                    nc.scalar.activation(ga[:], g[:], ACTF.Identity,
                                         scale=mask_sb[:, 0:1])
                    nc.scalar.activation(gb[:], g[:], ACTF.Identity,
                                         scale=mask_sb[:, 1:2])
                    for dcol in range(2):
                        nc.sync.dma_start(
                            at_in[dcol, utl],
                            ga[:, dcol * BS:(dcol + 1) * BS],
                        )
                        nc.sync.dma_start(
                            at_in[dcol, UT_HALF + utl],
                            gb[:, dcol * BS:(dcol + 1) * BS],
                        )

            def dend_exchange(layer, at_in, at_out):
                nc.gpsimd.collective_compute(
                    "ReduceScatter", ALU.add,
                    ins=[at_in.opt()],
                    outs=[at_out.opt()],
                    replica_groups=PAIRS,
                )
                for ut in range(UT_H):
                    nc.sync.dma_start(gateT[layer % 2][ut][:], at_out[ut])

            def h_apply(layer):
                bn_coeffs(layer)
                scale = scale_t[layer]
                nbias = nbias_t[layer]
                for ut in range(UT_H):
                    nc.scalar.activation(
                        h_tiles[ut][:], y_tiles[ut][:], ACTF.Relu,
                        bias=nbias[:, ut:ut + 1], scale=scale[:, ut:ut + 1],
                    )
                    nc.vector.tensor_tensor(
                        h_tiles[ut][:], h_tiles[ut][:],
                        gateT[layer % 2][ut][:], op=ALU.mult,
                    )

            at_bufs = []
            for L in range(3):
                at_in = dram.tile([2, UT_H, 128, BS], F16, tag=f"ati{L}",
                                  name=f"ati{L}")
                at_out = dram.tile([UT_H, 128, BS], F16,
                                   tag=f"ato{L}", name=f"ato{L}")
                at_bufs.append((at_in, at_out))

            # schedule: every collective is covered by an adjacent compute
            # phase except the last AllToAll (~20us before the classifier).
            w_phase(0)
            for k in range(KT_D):
                nc.sync.dma_start(ctx_tiles[k][:], ctxQ[k])
            dend_units(0, range(UT_HALF), at_bufs[0][0])
            dend_exchange(0, *at_bufs[0])
            dend_units(1, range(UT_HALF), at_bufs[1][0])
            dend_exchange(1, *at_bufs[1])
            h_apply(0)
            w_phase(1)
            dend_units(2, range(UT_HALF - 1), at_bufs[2][0])
            h_apply(1)
            w_phase(2)
            dend_units(2, [UT_HALF - 1], at_bufs[2][0])
            dend_exchange(2, *at_bufs[2])
            h_apply(2)

            # ---- classifier ----
            for ut in range(UT_O):
                wchunk = wpool.tile([128, KT_H * 128], F16, tag="wblk",
                                    name=f"wc_{ut}")
                nc.sync.dma_start(wchunk[:], wcr[ut].rearrange("p a b -> p (a b)"))
                ps = pb.tile([128, BS], F32, tag="yblk", name=f"cp{ut}")
                wv = wchunk[:]
                for kt in range(KT_H):
                    nc.tensor.matmul(
                        ps[:],
                        wv[:, kt * 128:(kt + 1) * 128],
                        h_tiles[kt][:],
                        start=(kt == 0),
                        stop=(kt == KT_H - 1),
                    )
                osb = opool.tile([128, BS], F32, tag="osb", name=f"osb{ut}")
                nc.scalar.activation(osb[:], ps[:], ACTF.Identity,
                                     bias=bc_sb[:, ut:ut + 1])
                nc.sync.dma_start(outT[ut], osb[:])

    nc.compile()
    return nc


def _prep_host(x, w0, b0, sw0, w1, b1, sw1, w2, b2, sw2, wc, bc):
    f = np.float32
    h16 = np.float16

    def _w_reorder(w, kt):  # w [H_out, K] -> [16ut, 128ki, kt, 128u]
        wT = np.ascontiguousarray(w.astype(h16).T)        # [K, H_out]
        K, HO = wT.shape
        return np.ascontiguousarray(
            wT.reshape(kt, 128, HO // 128, 128).transpose(2, 1, 0, 3))

    def _sw_reorder(sw):  # [H, S, D] -> [16ut, S, 128ki, 8kt, 128u]
        r = sw.astype(f).reshape(UT_H, 128, S, KT_D, 128)
        return np.ascontiguousarray(r.transpose(0, 2, 4, 3, 1))

    wc_pad = np.zeros((OUTP, H), f)
    wc_pad[:OUT] = wc.astype(f)
    bc_pad = np.zeros((OUTP,), f)
    bc_pad[:OUT] = bc.astype(f)

    common = {
        "wr0": _w_reorder(w0, KT_D),
        "wr1": _w_reorder(w1, KT_H),
        "wr2": _w_reorder(w2, KT_H),
        "wcr": _w_reorder(wc_pad, KT_H),
        "br": np.ascontiguousarray(
            np.stack([b0, b1, b2]).astype(f).reshape(3 * UT_H, 128).T),
        "bcr": np.ascontiguousarray(bc_pad.reshape(UT_O, 128).T),
    }
    # per unit-half segment weights (contiguous axis-0 slices, zero-copy)
    sw_half = []
    for swx in (sw0, sw1, sw2):
        full = _sw_reorder(swx)
        sw_half.append((full[:UT_HALF], full[UT_HALF:]))
    # per batch-quarter context, shared by core pairs
    ctx_q = []
    for i in range(4):
        xs = x[i * BQ:(i + 1) * BQ, D:]
        ctx_q.append(np.ascontiguousarray(xs.astype(f).T).reshape(KT_D, 128, BQ))

    in_maps = []
    for c in range(CORES):
        j = c & 1
        m = dict(common)
        mk = np.zeros((128, 2), f)
        mk[:, j] = 1.0
        m["dmask"] = mk
        m["xinT"] = np.ascontiguousarray(
            x[c * BS:(c + 1) * BS, :D].astype(h16).T).reshape(KT_D, 128, BS)
        m["ctxQ"] = ctx_q[c >> 1]
        for L in range(3):
            m[f"swr{L}"] = sw_half[L][j]
        in_maps.append(m)
    return in_maps


def kernel(**inputs):
    global LAST_EXEC_NS
    if "nc" not in _CACHE:
        _CACHE["nc"] = _build_nc()
    nc = _CACHE["nc"]

    in_maps = _prep_host(**inputs)

    trace = bool(int(os.environ.get("KERNEL_TRACE", "0")))
    if trace:
        _install_ntff_shim()

    tdir = None
    if trace:
        tdir = os.environ.get("KERNEL_TRACE_DIR")
        if tdir:
            os.makedirs(tdir, exist_ok=True)
    res = run_bass_kernel_spmd(nc, in_maps, core_ids=list(range(CORES)),
                               trace=trace, tmpdir=tdir)
    LAST_EXEC_NS = res.exec_time_ns

    out = np.empty((B, OUT), np.float32)
    for c in range(CORES):
        oT = res.results[c]["outT"].reshape(OUTP, BS)
        out[c * BS:(c + 1) * BS] = oT[:OUT].T
    return out
